# revision 1
# baseline (speedup 1.0000x reference)
"""Trainium2 Bass kernel for nn_ClassifierLSTM (2-layer masked LSTM classifier).

Strategy: tensor-parallel over the gate dimension across 8 NeuronCores.
Each core owns 128 hidden units (512 gate columns arranged [i|f|o|g]) of
both LSTM layers.  The two layer recurrences are interleaved (layer 2 runs
one step behind layer 1) so each step's hidden-state exchange is a single
AllGather of a [128, 128] bf16 tile (h1^T chunk | h2^T chunk).

The embedding lookup + layer-1 input projection are fused into a
premultiplied table  premult = emb @ k0_shard + b0_shard  ([32000, 512]
bf16 per core) built on-device; per step the 64 token rows are fetched
with an indirect (gather) DMA straight into the matmul moving operand.

Masking (mask_zero semantics) is folded into the i/f gate activations via
per-partition scale/bias on the Scalar engine, plus a select on h.
"""

import os
import sys

sys.path.insert(0, "/opt/trn_rl_repo")

import numpy as np
import ml_dtypes

import concourse.bass as bass
import concourse.mybir as mybir
import concourse.tile as tile
from concourse import bacc
from concourse.bass_utils import run_bass_kernel_spmd

F32 = mybir.dt.float32
BF16 = mybir.dt.bfloat16
I32 = mybir.dt.int32
AF = mybir.ActivationFunctionType
OP = mybir.AluOpType

VOCAB, EMB, HID, OUTD = 32000, 512, 1024, 3
B, T = 64, 512
NC = 8
SH = 512  # gate columns per core (= 4 * 128 hidden units)
HSH = 128  # hidden units per core
RB = 4  # bounce-buffer ring depth


def build(t_steps=T):
    nc = bacc.Bacc("TRN2", target_bir_lowering=False, debug=False, num_devices=NC)

    # ---- I/O ----
    tok = nc.dram_tensor("tok", [B, T], I32, kind="ExternalInput")
    embT = nc.dram_tensor("embT", [EMB, VOCAB], BF16, kind="ExternalInput")
    k0s = nc.dram_tensor("k0s", [EMB, SH], BF16, kind="ExternalInput")
    r0s = nc.dram_tensor("r0s", [HID, SH], BF16, kind="ExternalInput")
    k1s = nc.dram_tensor("k1s", [HID, SH], BF16, kind="ExternalInput")
    r1s = nc.dram_tensor("r1s", [HID, SH], BF16, kind="ExternalInput")
    b0s = nc.dram_tensor("b0s", [1, SH], BF16, kind="ExternalInput")
    b1s = nc.dram_tensor("b1s", [1, SH], BF16, kind="ExternalInput")
    wout = nc.dram_tensor("wout", [HID, OUTD], BF16, kind="ExternalInput")
    bout = nc.dram_tensor("bout", [1, OUTD], BF16, kind="ExternalInput")
    eye64 = nc.dram_tensor("eye64", [B, B], BF16, kind="ExternalInput")
    out = nc.dram_tensor("out", [B, OUTD], F32, kind="ExternalOutput")

    # ---- internal DRAM ----
    premult = nc.dram_tensor("premult", [VOCAB, SH], BF16)
    agin = [nc.dram_tensor(f"agin{k}", [HSH, 2 * B], BF16) for k in range(RB)]
    agout = [
        nc.dram_tensor(f"agout{k}", [NC * HSH, 2 * B], BF16, addr_space="Shared")
        for k in range(RB)
    ]

    KC0 = EMB // 128  # 4 K-chunks for the premult matmul
    KC = HID // 128  # 8 K-chunks for recurrent / layer-2 matmuls
    VT = VOCAB // 128  # 250 vocab tiles

    with tile.TileContext(nc) as tc:
        with (
            tc.tile_pool(name="persist", bufs=1) as pp,
            tc.tile_pool(name="wpool", bufs=1) as wp,
        ):
            # --- resident tiles ---
            tokS = pp.tile([B, T], I32)
            nc.sync.dma_start(tokS[:], tok[:])
            m = pp.tile([B, T], F32)
            # m = (tok != 0) as 1.0/0.0
            nc.vector.tensor_scalar(m[:], tokS[:], 0.0, None, OP.not_equal)
            biasI = pp.tile([B, T], F32)  # -30 where masked, 0 where valid
            nc.vector.tensor_scalar(biasI[:], m[:], 1.0, 30.0, OP.subtract, OP.mult)
            biasF = pp.tile([B, T], F32)  # +30 where masked, 0 where valid
            nc.vector.tensor_scalar(biasF[:], m[:], -30.0, 30.0, OP.mult, OP.add)

            eyeS = pp.tile([B, B], BF16)
            nc.sync.dma_start(eyeS[:], eye64[:])
            onesS = pp.tile([1, 128], BF16)
            nc.vector.memset(onesS[:], 1.0)
            b1S = pp.tile([1, SH], BF16)
            nc.sync.dma_start(b1S[:], b1s[:])
            boutS = pp.tile([1, OUTD], BF16)
            nc.sync.dma_start(boutS[:], bout[:])

            r0S = wp.tile([128, KC, SH], BF16)
            nc.sync.dma_start(r0S[:], r0s[:].rearrange("(c p) n -> p c n", p=128))
            k1S = wp.tile([128, KC, SH], BF16)
            nc.sync.dma_start(k1S[:], k1s[:].rearrange("(c p) n -> p c n", p=128))
            r1S = wp.tile([128, KC, SH], BF16)
            nc.sync.dma_start(r1S[:], r1s[:].rearrange("(c p) n -> p c n", p=128))
            k0S = wp.tile([128, KC0, SH], BF16)
            nc.sync.dma_start(k0S[:], k0s[:].rearrange("(c p) n -> p c n", p=128))
            woutS = wp.tile([128, KC, OUTD], BF16)
            nc.sync.dma_start(woutS[:], wout[:].rearrange("(c p) n -> p c n", p=128))
            b0S = pp.tile([1, SH], BF16)
            nc.sync.dma_start(b0S[:], b0s[:])

            # --- phase 1: premult = emb @ k0_shard + b0_shard ---
            with (
                tc.tile_pool(name="pm_sb", bufs=4) as pmsb,
                tc.tile_pool(name="pm_ps", bufs=2, space="PSUM") as pmps,
            ):
                for v in range(VT):
                    et = pmsb.tile([128, KC0, 128], BF16, tag="embtile")
                    nc.sync.dma_start(
                        et[:], embT[:, v * 128 : (v + 1) * 128].rearrange(
                            "(c p) n -> p c n", p=128
                        )
                    )
                    ps = pmps.tile([128, SH], F32)
                    nc.tensor.matmul(
                        ps[:], onesS[:1, :], b0S[:1, :], start=True, stop=False
                    )
                    for c in range(KC0):
                        nc.tensor.matmul(
                            ps[:], et[:, c, :], k0S[:, c, :],
                            start=False, stop=(c == KC0 - 1),
                        )
                    pv = pmsb.tile([128, SH], BF16, tag="pmtile")
                    nc.vector.tensor_copy(pv[:], ps[:])
                    nc.sync.dma_start(premult[v * 128 : (v + 1) * 128, :], pv[:])

            # --- phase 2: interleaved recurrences ---
            with (
                tc.tile_pool(name="state", bufs=2) as st,
                tc.tile_pool(name="gath", bufs=3) as gp,
                tc.tile_pool(name="gates", bufs=3) as gt,
                tc.tile_pool(name="xz", bufs=4) as xzp,
                tc.tile_pool(name="zps", bufs=2, space="PSUM") as zps,
                tc.tile_pool(name="trps", bufs=1, space="PSUM") as trps,
                tc.tile_pool(name="wps", bufs=1, space="PSUM") as wps,
            ):
                c1 = st.tile([B, HSH], F32, tag="c1")
                h1 = st.tile([B, HSH], BF16, tag="h1")
                c2 = st.tile([B, HSH], F32, tag="c2")
                h2 = st.tile([B, HSH], BF16, tag="h2")
                for tl in (c1, h1, c2, h2):
                    nc.vector.memset(tl[:], 0.0)

                hT1 = None  # gathered h1^T [128, KC, B] bf16
                hT2 = None  # gathered h2^T [128, KC, B] bf16

                def cell_update(z, li, t_idx, c_old, h_old):
                    """Gates+cell+mask for one layer step; returns (c_new, h_new, hTchunk_psum)."""
                    mcol = m[:, t_idx : t_idx + 1]
                    bI = biasI[:, t_idx : t_idx + 1]
                    bF = biasF[:, t_idx : t_idx + 1]
                    gi = gt.tile([B, HSH], F32, tag=f"gi{li}")
                    gf = gt.tile([B, HSH], F32, tag=f"gf{li}")
                    go = gt.tile([B, HSH], F32, tag=f"go{li}")
                    gg = gt.tile([B, HSH], F32, tag=f"gg{li}")
                    nc.scalar.activation(gg[:], z[:, 384:512], AF.Tanh)
                    nc.scalar.activation(gi[:], z[:, 0:128], AF.Sigmoid, bias=bI, scale=mcol)
                    nc.scalar.activation(gf[:], z[:, 128:256], AF.Sigmoid, bias=bF, scale=mcol)
                    nc.scalar.activation(go[:], z[:, 256:384], AF.Sigmoid)
                    u = gt.tile([B, HSH], F32, tag=f"u{li}")
                    nc.vector.tensor_tensor(u[:], gi[:], gg[:], OP.mult)
                    v = gt.tile([B, HSH], F32, tag=f"v{li}")
                    nc.vector.tensor_tensor(v[:], gf[:], c_old[:], OP.mult)
                    c_new = st.tile([B, HSH], F32, tag=f"c{li}")
                    nc.vector.tensor_tensor(c_new[:], u[:], v[:], OP.add)
                    th = gt.tile([B, HSH], F32, tag=f"th{li}")
                    nc.scalar.activation(th[:], c_new[:], AF.Tanh)
                    hn = gt.tile([B, HSH], F32, tag=f"hn{li}")
                    nc.vector.tensor_tensor(hn[:], go[:], th[:], OP.mult)
                    dh = gt.tile([B, HSH], F32, tag=f"dh{li}")
                    nc.vector.tensor_tensor(dh[:], hn[:], h_old[:], OP.subtract)
                    h_new = st.tile([B, HSH], BF16, tag=f"h{li}")
                    nc.vector.scalar_tensor_tensor(
                        h_new[:], dh[:], mcol, h_old[:], OP.mult, OP.add
                    )
                    trp = trps.tile([HSH, B], BF16, tag=f"tr{li}")
                    nc.tensor.transpose(trp[:], h_new[:], eyeS[:])
                    trs = gt.tile([HSH, B], BF16, tag=f"trs{li}")
                    nc.vector.tensor_copy(trs[:], trp[:])
                    return c_new, h_new, trs

                for t in range(t_steps + 1):
                    slot = t % RB
                    if t < t_steps:
                        # ---- layer-1 step t ----
                        gtile = xzp.tile([128, 1, SH], BF16, tag="xz1")
                        nc.gpsimd.indirect_dma_start(
                            out=gtile[:B, 0, :],
                            out_offset=None,
                            in_=premult[:],
                            in_offset=bass.IndirectOffsetOnAxis(
                                ap=tokS[:, t : t + 1], axis=0
                            ),
                        )
                        z1 = zps.tile([B, SH], F32, tag="z1")
                        # inject first: depends only on the gather, so the PE
                        # can run it while the AllGather is still in flight
                        # (keeps the HAM busy-window alive through the wait).
                        nc.tensor.matmul(
                            z1[:], eyeS[:], gtile[:B, 0, :],
                            start=True, stop=(t == 0),
                        )
                        if t > 0:
                            for c in range(KC):
                                nc.tensor.matmul(
                                    z1[:], hT1[:, c, :], r0S[:, c, :],
                                    start=False, stop=(c == KC - 1),
                                )
                        c1, h1, tr1 = cell_update(z1, 1, t, c1, h1)
                        nc.sync.dma_start(agin[slot][:, 0:B], tr1[:])

                    if t >= 1:
                        # ---- layer-2 step t-1 ----
                        s = t - 1
                        z2 = zps.tile([B, SH], F32, tag="z2")
                        nc.tensor.matmul(
                            z2[:], onesS[:1, :B], b1S[:1, :], start=True, stop=False
                        )
                        for c in range(KC):
                            nc.tensor.matmul(
                                z2[:], hT1[:, c, :], k1S[:, c, :],
                                start=False, stop=(s == 0 and c == KC - 1),
                            )
                        if s > 0:
                            for c in range(KC):
                                nc.tensor.matmul(
                                    z2[:], hT2[:, c, :], r1S[:, c, :],
                                    start=False, stop=(c == KC - 1),
                                )
                        c2, h2, tr2 = cell_update(z2, 2, s, c2, h2)
                        nc.sync.dma_start(agin[slot][:, B : 2 * B], tr2[:])
                    elif t == 0:
                        zb = gt.tile([HSH, B], BF16, tag="zb")
                        nc.vector.memset(zb[:], 0.0)
                        nc.sync.dma_start(agin[slot][:, B : 2 * B], zb[:])

                    # ---- exchange (one combined AllGather per slot) ----
                    nc.gpsimd.collective_compute(
                        "AllGather",
                        OP.bypass,
                        replica_groups=[list(range(NC))],
                        ins=[agin[slot][:].opt()],
                        outs=[agout[slot][:].opt()],
                    )
                    # split unbounce: layer-1 of the next slot only needs h1
                    hT1 = gp.tile([128, KC, B], BF16, tag="hT1")
                    nc.sync.dma_start(
                        hT1[:],
                        agout[slot][:, 0:B].rearrange("(c p) n -> p c n", p=128),
                    )
                    hT2 = gp.tile([128, KC, B], BF16, tag="hT2")
                    nc.sync.dma_start(
                        hT2[:],
                        agout[slot][:, B : 2 * B].rearrange("(c p) n -> p c n", p=128),
                    )

                # ---- output: logits = h2_final @ Wout + bout ----
                ops = zps.tile([B, OUTD], F32, tag="z1")
                nc.tensor.matmul(
                    ops[:], onesS[:1, :B], boutS[:1, :], start=True, stop=False
                )
                for c in range(KC):
                    nc.tensor.matmul(
                        ops[:], hT2[:, c, :], woutS[:, c, :],
                        start=False, stop=(c == KC - 1),
                    )
                logits = gt.tile([B, OUTD], F32, tag="logits")
                nc.vector.tensor_copy(logits[:], ops[:])
                nc.sync.dma_start(out[:], logits[:])

    nc.finalize()
    return nc


_CACHE = {}


def _get_nc(t_steps):
    if t_steps not in _CACHE:
        _CACHE[t_steps] = build(t_steps)
    return _CACHE[t_steps]


def _prep_in_maps(inputs, emb, k0, r0, b0, k1, r1, b1, Wout, bout):
    bf = ml_dtypes.bfloat16
    embT = np.ascontiguousarray(emb.T).astype(bf)
    tok = np.ascontiguousarray(inputs.astype(np.int32))
    eye = np.eye(B, dtype=bf)
    in_maps = []
    for c in range(NC):
        hc = slice(c * HSH, (c + 1) * HSH)
        # per-core gate-column permutation: [i | f | o | g] blocks
        cols = np.concatenate(
            [
                np.arange(0 * HID, 1 * HID)[hc],  # i
                np.arange(1 * HID, 2 * HID)[hc],  # f
                np.arange(3 * HID, 4 * HID)[hc],  # o
                np.arange(2 * HID, 3 * HID)[hc],  # g
            ]
        )
        in_maps.append(
            {
                "tok": tok,
                "embT": embT,
                "k0s": np.ascontiguousarray(k0[:, cols]).astype(bf),
                "r0s": np.ascontiguousarray(r0[:, cols]).astype(bf),
                "k1s": np.ascontiguousarray(k1[:, cols]).astype(bf),
                "r1s": np.ascontiguousarray(r1[:, cols]).astype(bf),
                "b0s": np.ascontiguousarray(b0[cols])[None, :].astype(bf),
                "b1s": np.ascontiguousarray(b1[cols])[None, :].astype(bf),
                "wout": np.ascontiguousarray(Wout).astype(bf),
                "bout": np.ascontiguousarray(bout)[None, :].astype(bf),
                "eye64": eye,
            }
        )
    return in_maps


def kernel(inputs, emb, k0, r0, b0, k1, r1, b1, Wout, bout, _trace=False):
    t_steps = int(os.environ.get("LSTM_T", T))
    nc = _get_nc(t_steps)
    in_maps = _prep_in_maps(
        np.asarray(inputs), np.asarray(emb), np.asarray(k0), np.asarray(r0),
        np.asarray(b0), np.asarray(k1), np.asarray(r1), np.asarray(b1),
        np.asarray(Wout), np.asarray(bout),
    )
    res = run_bass_kernel_spmd(
        nc, in_maps, core_ids=list(range(NC)), trace=_trace
    )
    kernel.last_result = res
    return res.results[0]["out"].astype(np.float32)

